# revision 3
# baseline (speedup 1.0000x reference)
"""Trainium2 Bass kernel for nn_DiversityMetric (batched NND diversity metric).

Math (per batch b, X = pred_poses[b] in R^{N x D}, N=2048, D=128):
    sq_dist[i,j] = ||xi||^2 + ||xj||^2 - 2 <xi,xj>, diag = inf
    nnd[i]       = sqrt(min_{j != i} sq_dist[i,j])
    out          = [mean(nnd), std(nnd, ddof=1), cv]   over all B*N points

v2 design (8 cores, 2 batches/core, bf16 matmul path):
  PE per row-strip m (128 rows x 2048 cols of the gram):
    - 4x [K=128, N=512] bf16 matmuls  v_ij = <xi, xj>         (start)
    - 1x identneg matmul: diag -= 1e6 (self-exclusion)
    - 4x row-tiled K=1 matmuls (tile_position=(32c,0), concurrent):
        v_ij += -0.5*sqn_j   -- lhsT = [-0.5] row, rhs = sqn_row chunk c
      sqn_row comes from 4 col-tiled K=128/M=1 matmuls per batch
      (tile_position=(0,32c)), so the per-column offset costs ~1/4 of
      the old full-array neghalf matmuls.
  Drain, split across both PSUM-capable engines (PATTERN per strip):
    'R': DVE tensor_reduce(max) per [128,1024] half -> rmax2[:,col,h]
    'S': ACT softmin: accum = sum_j exp(t*v_ij + bias_i) in one pass,
         bias_i = -t/2*(sqn_i - C).  Then max_j v_ij ~= (ln acc - bias)/t
         up to ~1e-4 (high-dim NN gaps make exp(t*(v-vmax)) tiny), and
         nnd_i^2 = sqn_i - 2*max = C - (2/t)*ln(acc): host-side one-liner.
  Host computes nnd2/sqrt/mean/std/cv from rmax2, lsum2, sqn (f64).
"""

import numpy as np
from contextlib import ExitStack

import ml_dtypes

import concourse.bass as bass
import concourse.bacc as bacc
import concourse.mybir as mybir
import concourse.tile as tile
from concourse.bass_utils import run_bass_kernel_spmd

F32 = mybir.dt.float32
BF16 = mybir.dt.bfloat16

B, N, D = 16, 2048, 128
NCORES = 8
BPC = B // NCORES          # batches per core
P = 128                    # partitions
MBLK = N // P              # 16 row strips per batch
MMW = 512                  # matmul moving width (1 PSUM bank)
CHUNK = 1024               # xt/sq SBUF chunk width
NEGBIG = -1.0e6

SOFT_T = 2.5               # softmin sharpness
SOFT_C = 160.0             # centering constant (~E[nnd^2]); exactly cancels

OFFS_TILED = True          # row-tiled K=1 offset matmuls (else neghalf K=128)
NSOFT = 13                 # strips drained by ACT softmin (rest: DVE reduce)

# Bresenham-interleaved S/R pattern over the 32 strips
PATTERN = [
    'S' if ((i + 1) * NSOFT) // (BPC * MBLK) > (i * NSOFT) // (BPC * MBLK)
    else 'R'
    for i in range(BPC * MBLK)
]

_CACHE = {}


def build_kernel():
    nc = bacc.Bacc("TRN2", target_bir_lowering=False, debug=False)

    xt_d = nc.dram_tensor("xt", [BPC, P, N], BF16, kind="ExternalInput")
    neghalf_d = nc.dram_tensor("neghalf", [P, P], BF16, kind="ExternalInput")
    ones_d = nc.dram_tensor("onescol", [P, 2], BF16, kind="ExternalInput")
    identneg_d = nc.dram_tensor("identneg", [P, P], BF16, kind="ExternalInput")
    ident_d = nc.dram_tensor("ident", [P, P], BF16, kind="ExternalInput")
    rmax2_d = nc.dram_tensor("rmax2", [P, BPC * MBLK, 2], F32,
                             kind="ExternalOutput")
    lsum2_d = nc.dram_tensor("lsum2", [P, BPC * MBLK, 2], F32,
                             kind="ExternalOutput")
    sqn_d = nc.dram_tensor("sqn", [P, BPC * MBLK], F32, kind="ExternalOutput")

    with tile.TileContext(nc) as tc, ExitStack() as ctx:
        const = ctx.enter_context(tc.tile_pool(name="const", bufs=1))
        xpool = ctx.enter_context(tc.tile_pool(name="x", bufs=1))
        spool = ctx.enter_context(tc.tile_pool(name="s", bufs=1))
        small = ctx.enter_context(tc.tile_pool(name="small", bufs=1))
        rowp = ctx.enter_context(tc.tile_pool(name="rowp", bufs=1))
        junkp = ctx.enter_context(tc.tile_pool(name="junk", bufs=1))
        psum = ctx.enter_context(tc.tile_pool(name="psum", bufs=3, space="PSUM"))
        psq = ctx.enter_context(tc.tile_pool(name="psq", bufs=1, space="PSUM"))

        NCHUNK = N // CHUNK
        xts = {}
        sqs = {}

        def load_chunk(b, c):
            xtile = xpool.tile([P, CHUNK], BF16, tag=f"xt_{b}_{c}")
            nc.sync.dma_start(
                xtile[:], xt_d.ap()[b, :, c * CHUNK:(c + 1) * CHUNK]
            )
            xts[(b, c)] = xtile
            stile = spool.tile([P, CHUNK], BF16, tag=f"sq_{b}_{c}")
            nc.gpsimd.tensor_mul(stile[:], xtile[:], xtile[:])
            sqs[(b, c)] = stile

        load_chunk(0, 0)

        neghalf = const.tile([P, P], BF16)
        nc.scalar.dma_start(neghalf[:], neghalf_d.ap())
        onescol = const.tile([P, 2], BF16)
        nc.scalar.dma_start(onescol[:], ones_d.ap())
        identneg = const.tile([P, P], BF16)
        nc.scalar.dma_start(identneg[:], identneg_d.ap())
        ident = const.tile([P, P], BF16)
        nc.scalar.dma_start(ident[:], ident_d.ap())

        for b in range(BPC):
            for c in range(NCHUNK):
                if (b, c) != (0, 0):
                    load_chunk(b, c)

        def xcol(b, j0, w):
            c = j0 // CHUNK
            off = j0 - c * CHUNK
            assert off + w <= CHUNK
            return xts[(b, c)][:, off:off + w]

        def scol(b, j0, w):
            c = j0 // CHUNK
            off = j0 - c * CHUNK
            assert off + w <= CHUNK
            return sqs[(b, c)][:, off:off + w]

        rmax2 = small.tile([P, BPC * MBLK, 2], F32)
        nc.gpsimd.memset(rmax2[:], -1.0e30)
        lsum2 = small.tile([P, BPC * MBLK, 2], F32)
        nc.gpsimd.memset(lsum2[:], 0.0)
        sqn_sb = small.tile([P, BPC * MBLK], F32)
        bias_sb = small.tile([P, BPC * MBLK], F32)
        junk = junkp.tile([P, CHUNK], BF16)

        sqnrows = {}

        def batch_setup(b):
            # sqn_row: 4 col-tiled K=128/M=1 matmuls -> psqS partitions 0/32/64/96
            psqS = psq.tile([P, MMW], F32, tag="psqS")
            nc.vector.memset(psqS[:], 0.0)
            for c in range(4):
                nc.tensor.matmul(
                    psqS[32 * c:32 * c + 1, :],
                    onescol[:, 0:1],
                    scol(b, c * MMW, MMW),
                    start=True, stop=True,
                    tile_position=(0, 32 * c),
                )
            srow = rowp.tile([P, MMW], BF16, tag=f"sqnrow_{b}")
            nc.vector.tensor_copy(srow[:], psqS[:])
            sqnrows[b] = srow

            # per-point sqn columns: 16 full-mode matmuls reusing psqS cols
            for m in range(MBLK):
                nc.tensor.matmul(
                    psqS[:, 2 * m:2 * m + 2],
                    scol(b, m * P, P),
                    onescol[:],
                    start=True, stop=True,
                )
            nc.vector.tensor_copy(
                sqn_sb[:, b * MBLK:(b + 1) * MBLK],
                psqS[:, 0:2 * MBLK].rearrange(
                    "p (c t) -> p c t", t=2)[:, :, 0:1],
            )
            # bias_i = -t/2*(sqn_i - C)
            nc.vector.tensor_scalar(
                bias_sb[:, b * MBLK:(b + 1) * MBLK],
                sqn_sb[:, b * MBLK:(b + 1) * MBLK],
                -0.5 * SOFT_T, 0.5 * SOFT_T * SOFT_C,
                op0=mybir.AluOpType.mult, op1=mybir.AluOpType.add,
            )

        for b in range(BPC):
            batch_setup(b)
            for m in range(MBLK):
                col = b * MBLK + m
                path = PATTERN[col]
                lhs_x = xcol(b, m * P, P)
                phs = []
                for h in range(2):
                    ph = psum.tile([P, N // 2], F32, tag="ph")
                    phs.append(ph)
                    for k in range(2):
                        j0 = h * (N // 2) + k * MMW
                        nc.tensor.matmul(
                            ph[:, k * MMW:(k + 1) * MMW],
                            lhs_x,
                            xcol(b, j0, MMW),
                            start=True, stop=False,
                        )
                hd = (m * P) // (N // 2)
                off = m * P - hd * (N // 2)
                nc.tensor.matmul(
                    phs[hd][:, off:off + P],
                    identneg[:],
                    ident[:],
                    start=False, stop=False,
                )
                if OFFS_TILED:
                    for h in range(2):
                        for k in range(2):
                            c = 2 * h + k
                            nc.tensor.matmul(
                                phs[h][:, k * MMW:(k + 1) * MMW],
                                neghalf[32 * c:32 * c + 1, :],
                                sqnrows[b][32 * c:32 * c + 1, :],
                                start=False, stop=True,
                                tile_position=(32 * c, 0),
                            )
                else:
                    for h in range(2):
                        for k in range(2):
                            j0 = h * (N // 2) + k * MMW
                            nc.tensor.matmul(
                                phs[h][:, k * MMW:(k + 1) * MMW],
                                neghalf[:],
                                scol(b, j0, MMW),
                                start=False, stop=True,
                            )
                if path == 'R':
                    for h in range(2):
                        nc.vector.tensor_reduce(
                            rmax2[:, col, h:h + 1], phs[h][:],
                            axis=mybir.AxisListType.X, op=mybir.AluOpType.max,
                        )
                else:
                    for h in range(2):
                        nc.scalar.activation(
                            junk[:], phs[h][:],
                            mybir.ActivationFunctionType.Exp,
                            bias=bias_sb[:, col:col + 1],
                            scale=SOFT_T,
                            accum_out=lsum2[:, col, h:h + 1],
                        )

        nc.sync.dma_start(rmax2_d.ap()[:, :, :], rmax2[:])
        nc.sync.dma_start(lsum2_d.ap()[:, :, :], lsum2[:])
        nc.sync.dma_start(sqn_d.ap()[:, :], sqn_sb[:])

    nc.compile()
    return nc


def _consts():
    neghalf = np.full((P, P), -0.5, dtype=ml_dtypes.bfloat16)
    onescol = np.ones((P, 2), dtype=ml_dtypes.bfloat16)
    identneg = (NEGBIG * np.eye(P)).astype(ml_dtypes.bfloat16)
    ident = np.eye(P, dtype=np.float32).astype(ml_dtypes.bfloat16)
    return neghalf, onescol, identneg, ident


def make_in_maps(pred_poses):
    neghalf, onescol, identneg, ident = _consts()
    in_maps = []
    for c in range(NCORES):
        xb = pred_poses[c * BPC:(c + 1) * BPC]
        xt = np.ascontiguousarray(
            xb.transpose(0, 2, 1)).astype(ml_dtypes.bfloat16)
        in_maps.append({
            "xt": xt, "neghalf": neghalf, "onescol": onescol,
            "identneg": identneg, "ident": ident,
        })
    return in_maps


def postprocess(rmax2, lsum2, sqn):
    """[128,32,2],[128,32,2],[128,32] (one core) -> nnd2 [128,32] (f64)."""
    rmax2 = np.asarray(rmax2, dtype=np.float64)
    lsum2 = np.asarray(lsum2, dtype=np.float64)
    sqn = np.asarray(sqn, dtype=np.float64)
    nnd2 = np.empty((P, BPC * MBLK), dtype=np.float64)
    for col in range(BPC * MBLK):
        if PATTERN[col] == 'R':
            rmax = np.maximum(rmax2[:, col, 0], rmax2[:, col, 1])
            nnd2[:, col] = sqn[:, col] - 2.0 * rmax
        else:
            lsum = lsum2[:, col, 0] + lsum2[:, col, 1]
            nnd2[:, col] = SOFT_C - (2.0 / SOFT_T) * np.log(
                np.maximum(lsum, 1e-300))
    return np.maximum(nnd2, 0.0)


def kernel(pred_poses: np.ndarray) -> np.ndarray:
    pred_poses = np.ascontiguousarray(np.asarray(pred_poses, dtype=np.float32))
    assert pred_poses.shape == (B, N, D)

    if "nc" not in _CACHE:
        _CACHE["nc"] = build_kernel()
    nc = _CACHE["nc"]

    in_maps = make_in_maps(pred_poses)
    res = run_bass_kernel_spmd(nc, in_maps, list(range(NCORES)))

    nnd = np.zeros((B, N), dtype=np.float64)
    for c in range(NCORES):
        r = res.results[c]
        nnd2 = postprocess(r["rmax2"], r["lsum2"], r["sqn"])
        t = np.sqrt(nnd2)                               # [128, 32]
        for bl in range(BPC):
            sub = t[:, bl * MBLK:(bl + 1) * MBLK]       # [128, 16] (p, m)
            nnd[c * BPC + bl] = sub.T.reshape(N)        # index m*128+p

    mean = nnd.mean()
    std = nnd.std(ddof=1)
    eps = 1e-8
    cv = std / max(mean, eps) if mean > eps else 0.0
    return np.stack([mean, std, cv]).astype(np.float32)


# revision 4
# speedup vs baseline: 1.0777x; 1.0777x over previous
"""Trainium2 Bass kernel for nn_DiversityMetric (batched NND diversity metric).

Math (per batch b, X = pred_poses[b] in R^{N x D}, N=2048, D=128):
    sq_dist[i,j] = ||xi||^2 + ||xj||^2 - 2 <xi,xj>, diag = inf
    nnd[i]       = sqrt(min_{j != i} sq_dist[i,j])
    out          = [mean(nnd), std(nnd, ddof=1), cv]   over all B*N points

v3 design (8 cores, 2 batches/core, bf16 matmul path):
  PE per row-strip m (128 rows x 2048 cols of the gram):
    - 4x [K=128, N=512] bf16 matmuls  v_ij = <xi, xj>         (start)
    - 1x identneg matmul: diag -= 1e6 (self-exclusion)
    - 4x row-tiled K=1 matmuls (tile_position=(32c,0), concurrent):
        v_ij += -0.5*sqn_j   -- lhsT = [-0.5] row, rhs = sqn_row chunk c
  Batch setup (borrows one PSUM slot):
    - sqn_row via 4 col-tiled K=128/M=32 matmuls (tile_position=(0,32c))
    - per-point sqn columns via 16 tiny K=128/N=2 matmuls
  Drain, strictly alternating across both PSUM-capable engines:
    'R': DVE tensor_reduce(max) per [128,1024] half -> rmax2[:,col,h]
    'S': ACT softmin: accum = sum_j exp(t*v_ij + bias_i) in one pass,
         bias_i = -t/2*(sqn_i - C).  max_j v_ij ~= (ln acc - bias)/t
         (high-dim NN gaps make exp(t*(v-vmax)) tiny), so
         nnd_i^2 = sqn_i - 2*max = C - (2/t)*ln(acc): host-side.
  PSUM: single pool of 4x [128,1024] slots (8 banks) so PE always has
  two free halves per strip -- avoids PE idle gaps (HAM p-state drops).
  Host computes nnd2/sqrt/mean/std/cv from rmax2, lsum2, sqn (f64).
"""

import numpy as np
from contextlib import ExitStack

import ml_dtypes

import concourse.bass as bass
import concourse.bacc as bacc
import concourse.mybir as mybir
import concourse.tile as tile
from concourse.bass_utils import run_bass_kernel_spmd

F32 = mybir.dt.float32
BF16 = mybir.dt.bfloat16

B, N, D = 16, 2048, 128
NCORES = 8
BPC = B // NCORES          # batches per core
P = 128                    # partitions
MBLK = N // P              # 16 row strips per batch
MMW = 512                  # matmul moving width (1 PSUM bank)
CHUNK = 1024               # xt/sq SBUF chunk width
NEGBIG = -1.0e6

SOFT_T = 2.5               # softmin sharpness
SOFT_C = 160.0             # centering constant (~E[nnd^2]); exactly cancels

OFFS_TILED = True          # row-tiled K=1 offset matmuls (else neghalf K=128)
NSOFT = 15                 # strips drained by ACT softmin (rest: DVE reduce)

# Bresenham-interleaved S/R pattern over the 32 strips (near-alternating)
PATTERN = [
    'S' if ((i + 1) * NSOFT) // (BPC * MBLK) > (i * NSOFT) // (BPC * MBLK)
    else 'R'
    for i in range(BPC * MBLK)
]

_CACHE = {}


def build_kernel():
    nc = bacc.Bacc("TRN2", target_bir_lowering=False, debug=False)

    xt_d = nc.dram_tensor("xt", [BPC, P, N], BF16, kind="ExternalInput")
    neghalf_d = nc.dram_tensor("neghalf", [P, P], BF16, kind="ExternalInput")
    ones_d = nc.dram_tensor("ones32", [P, 32], BF16, kind="ExternalInput")
    identneg_d = nc.dram_tensor("identneg", [P, P], BF16, kind="ExternalInput")
    ident_d = nc.dram_tensor("ident", [P, P], BF16, kind="ExternalInput")
    rmax2_d = nc.dram_tensor("rmax2", [P, BPC * MBLK, 2], F32,
                             kind="ExternalOutput")
    lsum2_d = nc.dram_tensor("lsum2", [P, BPC * MBLK, 2], F32,
                             kind="ExternalOutput")
    sqn_d = nc.dram_tensor("sqn", [P, BPC * MBLK], F32, kind="ExternalOutput")

    with tile.TileContext(nc) as tc, ExitStack() as ctx:
        const = ctx.enter_context(tc.tile_pool(name="const", bufs=1))
        xpool = ctx.enter_context(tc.tile_pool(name="x", bufs=1))
        spool = ctx.enter_context(tc.tile_pool(name="s", bufs=1))
        small = ctx.enter_context(tc.tile_pool(name="small", bufs=1))
        rowp = ctx.enter_context(tc.tile_pool(name="rowp", bufs=1))
        junkp = ctx.enter_context(tc.tile_pool(name="junk", bufs=1))
        psum = ctx.enter_context(tc.tile_pool(name="psum", bufs=4, space="PSUM"))

        NCHUNK = N // CHUNK
        xts = {}
        sqs = {}

        def load_chunk(b, c):
            xtile = xpool.tile([P, CHUNK], BF16, tag=f"xt_{b}_{c}")
            nc.sync.dma_start(
                xtile[:], xt_d.ap()[b, :, c * CHUNK:(c + 1) * CHUNK]
            )
            xts[(b, c)] = xtile
            stile = spool.tile([P, CHUNK], BF16, tag=f"sq_{b}_{c}")
            if b == 0:
                nc.scalar.square(stile[:], xtile[:])
            else:
                nc.gpsimd.tensor_mul(stile[:], xtile[:], xtile[:])
            sqs[(b, c)] = stile

        load_chunk(0, 0)

        neghalf = const.tile([P, P], BF16)
        nc.scalar.dma_start(neghalf[:], neghalf_d.ap())
        ones32 = const.tile([P, 32], BF16)
        nc.scalar.dma_start(ones32[:], ones_d.ap())
        identneg = const.tile([P, P], BF16)
        nc.scalar.dma_start(identneg[:], identneg_d.ap())
        ident = const.tile([P, P], BF16)
        nc.scalar.dma_start(ident[:], ident_d.ap())

        for b in range(BPC):
            for c in range(NCHUNK):
                if (b, c) != (0, 0):
                    load_chunk(b, c)

        def xcol(b, j0, w):
            c = j0 // CHUNK
            off = j0 - c * CHUNK
            assert off + w <= CHUNK
            return xts[(b, c)][:, off:off + w]

        def scol(b, j0, w):
            c = j0 // CHUNK
            off = j0 - c * CHUNK
            assert off + w <= CHUNK
            return sqs[(b, c)][:, off:off + w]

        rmax2 = small.tile([P, BPC * MBLK, 2], F32)
        nc.gpsimd.memset(rmax2[:], -1.0e30)
        lsum2 = small.tile([P, BPC * MBLK, 2], F32)
        nc.gpsimd.memset(lsum2[:], 0.0)
        sqn_sb = small.tile([P, BPC * MBLK], F32)
        bias_sb = small.tile([P, BPC * MBLK], F32)
        junk = junkp.tile([P, CHUNK], BF16)

        sqnrows = {}

        def batch_setup(b):
            # borrow one psum slot for this batch's setup matmuls
            ps = psum.tile([P, CHUNK], F32, tag="ph")
            # sqn_row: 4 col-tiled K=128/M=32 matmuls -> all 128 partitions
            for c in range(4):
                nc.tensor.matmul(
                    ps[32 * c:32 * c + 32, 0:MMW],
                    ones32[:],
                    scol(b, c * MMW, MMW),
                    start=True, stop=True,
                    tile_position=(0, 32 * c),
                )
            # per-point sqn columns: 16 tiny matmuls into cols [512, 544)
            for m in range(MBLK):
                nc.tensor.matmul(
                    ps[:, MMW + 2 * m:MMW + 2 * m + 2],
                    scol(b, m * P, P),
                    ones32[:, 0:2],
                    start=True, stop=True,
                )
            srow = rowp.tile([P, MMW], BF16, tag=f"sqnrow_{b}")
            nc.vector.tensor_copy(srow[:], ps[:, 0:MMW])
            sqnrows[b] = srow
            nc.vector.tensor_copy(
                sqn_sb[:, b * MBLK:(b + 1) * MBLK],
                ps[:, MMW:MMW + 2 * MBLK].rearrange(
                    "p (c t) -> p c t", t=2)[:, :, 0:1],
            )
            # bias_i = -t/2*(sqn_i - C)
            nc.vector.tensor_scalar(
                bias_sb[:, b * MBLK:(b + 1) * MBLK],
                sqn_sb[:, b * MBLK:(b + 1) * MBLK],
                -0.5 * SOFT_T, 0.5 * SOFT_T * SOFT_C,
                op0=mybir.AluOpType.mult, op1=mybir.AluOpType.add,
            )

        for b in range(BPC):
            batch_setup(b)
            for m in range(MBLK):
                col = b * MBLK + m
                path = PATTERN[col]
                lhs_x = xcol(b, m * P, P)
                phs = []
                for h in range(2):
                    ph = psum.tile([P, N // 2], F32, tag="ph")
                    phs.append(ph)
                    for k in range(2):
                        j0 = h * (N // 2) + k * MMW
                        nc.tensor.matmul(
                            ph[:, k * MMW:(k + 1) * MMW],
                            lhs_x,
                            xcol(b, j0, MMW),
                            start=True, stop=False,
                        )
                hd = (m * P) // (N // 2)
                off = m * P - hd * (N // 2)
                nc.tensor.matmul(
                    phs[hd][:, off:off + P],
                    identneg[:],
                    ident[:],
                    start=False, stop=False,
                )
                if OFFS_TILED:
                    for h in range(2):
                        for k in range(2):
                            c = 2 * h + k
                            nc.tensor.matmul(
                                phs[h][:, k * MMW:(k + 1) * MMW],
                                neghalf[32 * c:32 * c + 1, :],
                                sqnrows[b][32 * c:32 * c + 1, :],
                                start=False, stop=True,
                                tile_position=(32 * c, 0),
                            )
                else:
                    for h in range(2):
                        for k in range(2):
                            j0 = h * (N // 2) + k * MMW
                            nc.tensor.matmul(
                                phs[h][:, k * MMW:(k + 1) * MMW],
                                neghalf[:],
                                scol(b, j0, MMW),
                                start=False, stop=True,
                            )
                if path == 'R':
                    for h in range(2):
                        nc.vector.tensor_reduce(
                            rmax2[:, col, h:h + 1], phs[h][:],
                            axis=mybir.AxisListType.X, op=mybir.AluOpType.max,
                        )
                else:
                    for h in range(2):
                        nc.scalar.activation(
                            junk[:], phs[h][:],
                            mybir.ActivationFunctionType.Exp,
                            bias=bias_sb[:, col:col + 1],
                            scale=SOFT_T,
                            accum_out=lsum2[:, col, h:h + 1],
                        )

        nc.sync.dma_start(rmax2_d.ap()[:, :, :], rmax2[:])
        nc.sync.dma_start(lsum2_d.ap()[:, :, :], lsum2[:])
        nc.sync.dma_start(sqn_d.ap()[:, :], sqn_sb[:])

    nc.compile()
    return nc


def _consts():
    neghalf = np.full((P, P), -0.5, dtype=ml_dtypes.bfloat16)
    ones32 = np.ones((P, 32), dtype=ml_dtypes.bfloat16)
    identneg = (NEGBIG * np.eye(P)).astype(ml_dtypes.bfloat16)
    ident = np.eye(P, dtype=np.float32).astype(ml_dtypes.bfloat16)
    return neghalf, ones32, identneg, ident


def make_in_maps(pred_poses):
    neghalf, ones32, identneg, ident = _consts()
    in_maps = []
    for c in range(NCORES):
        xb = pred_poses[c * BPC:(c + 1) * BPC]
        xt = np.ascontiguousarray(
            xb.transpose(0, 2, 1)).astype(ml_dtypes.bfloat16)
        in_maps.append({
            "xt": xt, "neghalf": neghalf, "ones32": ones32,
            "identneg": identneg, "ident": ident,
        })
    return in_maps


def postprocess(rmax2, lsum2, sqn):
    """[128,32,2],[128,32,2],[128,32] (one core) -> nnd2 [128,32] (f64)."""
    rmax2 = np.asarray(rmax2, dtype=np.float64)
    lsum2 = np.asarray(lsum2, dtype=np.float64)
    sqn = np.asarray(sqn, dtype=np.float64)
    nnd2 = np.empty((P, BPC * MBLK), dtype=np.float64)
    for col in range(BPC * MBLK):
        if PATTERN[col] == 'R':
            rmax = np.maximum(rmax2[:, col, 0], rmax2[:, col, 1])
            nnd2[:, col] = sqn[:, col] - 2.0 * rmax
        else:
            lsum = lsum2[:, col, 0] + lsum2[:, col, 1]
            nnd2[:, col] = SOFT_C - (2.0 / SOFT_T) * np.log(
                np.maximum(lsum, 1e-300))
    return np.maximum(nnd2, 0.0)


def kernel(pred_poses: np.ndarray) -> np.ndarray:
    pred_poses = np.ascontiguousarray(np.asarray(pred_poses, dtype=np.float32))
    assert pred_poses.shape == (B, N, D)

    if "nc" not in _CACHE:
        _CACHE["nc"] = build_kernel()
    nc = _CACHE["nc"]

    in_maps = make_in_maps(pred_poses)
    res = run_bass_kernel_spmd(nc, in_maps, list(range(NCORES)))

    nnd = np.zeros((B, N), dtype=np.float64)
    for c in range(NCORES):
        r = res.results[c]
        nnd2 = postprocess(r["rmax2"], r["lsum2"], r["sqn"])
        t = np.sqrt(nnd2)                               # [128, 32]
        for bl in range(BPC):
            sub = t[:, bl * MBLK:(bl + 1) * MBLK]       # [128, 16] (p, m)
            nnd[c * BPC + bl] = sub.T.reshape(N)        # index m*128+p

    mean = nnd.mean()
    std = nnd.std(ddof=1)
    eps = 1e-8
    cv = std / max(mean, eps) if mean > eps else 0.0
    return np.stack([mean, std, cv]).astype(np.float32)
